# revision 1
# baseline (speedup 1.0000x reference)
"""GQA attention with QK-norm for Trainium2, sharded over 8 NeuronCores.

Problem: B=2, T=2048, D=2048, H=16 query heads, KVH=4 kv heads, dk=128.
    Q = q @ Wq.T ; K = k @ Wk.T ; V = v @ Wv.T  (per batch)
    Q = g * l2norm(Q, per head) ; K = l2norm(K, per head)
    out = softmax(causal(Q K^T / sqrt(dk))) V @ Wo.T

Sharding: core c = 4*b + gi handles batch b and kv-head group gi
(4 query heads + 1 kv head). Each core computes a row-shard of the
output projection (O^T partial over its 512 head-dims); the host sums
the 4 partials per batch. No device collectives.

On-core layout is feature-major ("transposed") throughout:
  activations arrive as q^T/k^T/v^T (host pre-tiled so every DMA moves
  contiguous 4-16KB rows); projections produce Q^T/K^T/V^T [dk, T];
  scores are computed directly as S^T[k, q] = (K^T).T @ Q^T; softmax
  numerator exp(S^T) needs no max-subtraction because QK-norm bounds
  |scores| <= g/sqrt(dk) ~ 0.674. Attention runs two-stage per
  (head, q-block): stage 1 streams S^T->exp into an SBUF strip
  (ACT-bound), stage 2 consumes the strip with dense back-to-back
  ones-rowsum and Y^T matmuls (PE-bound); the stages pipeline across
  heads. Row-sum reciprocals are applied to Y^T via a
  partition-broadcast DMA.
"""

import math
import os
import sys

for _p in ("/opt/trn_rl_repo",):
    if _p not in sys.path:
        sys.path.append(_p)

import numpy as np
from concourse import bacc, mybir, tile
from concourse.bass_utils import run_bass_kernel_spmd
from concourse.masks import make_identity

B, T, D, H, KVH, DK = 2, 2048, 2048, 16, 4, 128
HPG = H // KVH          # query heads per core (group)
E = HPG * DK            # 512: q-head dims per core
P = 128
TB = 4                  # t blocks of 512
NT = T // P             # 16 tiles of 128 along T
ND = D // P             # 16 contraction tiles
f32 = mybir.dt.float32
f32r = mybir.dt.float32r
bf16 = mybir.dt.bfloat16
AF = mybir.ActivationFunctionType
EPS2 = 1e-24

MM_DT = {"f32r": f32r, "bf16": bf16, "f32": f32}[
    os.environ.get("ATTN_DT", "bf16")]
ST_DT = bf16 if MM_DT == bf16 else f32


def _ld(x):
    return x.bitcast(MM_DT) if MM_DT is f32r else x


def build_kernel():
    nc = bacc.Bacc(None, target_bir_lowering=False)

    # host-pre-tiled inputs (see make_in_maps): every DMA below reads
    # contiguous multi-KB rows.
    qTt = nc.declare_dram_parameter("qTt", [TB, P, ND * 512], ST_DT,
                                    isOutput=False)
    kT = nc.declare_dram_parameter("kT", [D, T], ST_DT, isOutput=False)
    vT = nc.declare_dram_parameter("vT", [D, T], ST_DT, isOutput=False)
    wqt = nc.declare_dram_parameter("wqt", [P, ND * E], ST_DT, isOutput=False)
    wkt = nc.declare_dram_parameter("wkt", [P, ND * DK], ST_DT,
                                    isOutput=False)
    wvt = nc.declare_dram_parameter("wvt", [P, ND * DK], ST_DT,
                                    isOutput=False)
    wot = nc.declare_dram_parameter("wot", [P, HPG * D], ST_DT,
                                    isOutput=False)
    gs16 = nc.declare_dram_parameter("gs16", [NT, HPG], f32, isOutput=False)
    outT = nc.declare_dram_parameter("outT", [D, T], f32, isOutput=True)

    n2_dram = nc.dram_tensor("n2_dram", [HPG + 1, T], f32)
    c_dram = nc.dram_tensor("c_dram", [HPG + 1, T], f32)
    inv_dram = nc.dram_tensor("inv_dram", [HPG, T], f32)

    from contextlib import ExitStack

    with tile.TileContext(nc) as tc:
        with ExitStack() as outer:
            const = outer.enter_context(tc.tile_pool(name="const", bufs=1))
            persist = outer.enter_context(tc.tile_pool(name="persist", bufs=1))

            ident = const.tile([P, P], f32, tag="ident")
            make_identity(nc, ident[:])
            ones_f32 = const.tile([P, 1], f32, tag="ones_f32")
            nc.vector.memset(ones_f32[:], 1.0)
            ones = const.tile([P, 1], MM_DT, tag="ones")
            nc.vector.tensor_copy(ones[:], ones_f32[:])
            gs_sb = const.tile([NT, HPG], f32, tag="gs")
            nc.sync.dma_start(gs_sb[:], gs16[:])
            eps16 = const.tile([NT, 1], f32, tag="eps16")
            nc.vector.memset(eps16[:], EPS2)
            # causal keep-mask: M[p, c] = 1.0 iff c >= p + 384.
            # diagonal k-tile j (0..3) of a 512-wide q block uses
            # M[:, 384-128j : 896-128j]  ==  1{ f >= p + 128 j }.
            maskM = const.tile([P, 896], f32, tag="mask")
            nc.vector.memset(maskM[:], 1.0)
            nc.gpsimd.affine_select(
                out=maskM[:], in_=maskM[:],
                compare_op=mybir.AluOpType.is_ge,
                fill=0.0, base=-384,
                pattern=[[1, 896]], channel_multiplier=-1,
            )

            qt_sb = persist.tile([P, HPG * T], MM_DT, tag="qt")
            kt_sb = persist.tile([P, T], MM_DT, tag="kt")
            vtm_sb = persist.tile([P, T], MM_DT, tag="vtm")
            yt_sb = persist.tile([P, HPG * T], MM_DT, tag="yt")

            # ---------------- phase A: projections + norms ----------------
            with ExitStack() as pa:
                wpool = pa.enter_context(tc.tile_pool(name="wpool", bufs=1))
                actsq = pa.enter_context(tc.tile_pool(name="actsq", bufs=4))
                actskv = pa.enter_context(tc.tile_pool(name="actskv", bufs=3))
                scratch = pa.enter_context(tc.tile_pool(name="scratch",
                                                        bufs=2))
                bcast = pa.enter_context(tc.tile_pool(name="bcast", bufs=2))
                rows = pa.enter_context(tc.tile_pool(name="rows", bufs=4))
                smal = pa.enter_context(tc.tile_pool(name="smal", bufs=3))
                psA = pa.enter_context(
                    tc.tile_pool(name="psA", bufs=6, space="PSUM"))
                psTP = pa.enter_context(
                    tc.tile_pool(name="psTP", bufs=2, space="PSUM"))

                wv_sb = wpool.tile([P, ND * DK], MM_DT, tag="wv")
                nc.sync.dma_start(wv_sb[:], _ld(wvt[:]))
                wq_sb = wpool.tile([P, ND * E], MM_DT, tag="wq")
                nc.sync.dma_start(wq_sb[:], _ld(wqt[:]))

                def proj_kv(src_dram, w_sb, dst_fn):
                    accs = [psA.tile([P, 512], f32, tag="proj",
                                     name=f"acc{_t}") for _t in range(TB)]
                    for n in range(ND):
                        a = actskv.tile([P, T], MM_DT, tag="akv")
                        nc.sync.dma_start(
                            a[:], _ld(src_dram[n * P:(n + 1) * P, :]))
                        for tb in range(TB):
                            nc.tensor.matmul(
                                accs[tb][:],
                                w_sb[:, n * DK:(n + 1) * DK],
                                a[:, tb * 512:(tb + 1) * 512],
                                start=(n == 0), stop=(n == ND - 1))
                    for tb in range(TB):
                        dst_fn(tb, accs[tb])

                vt_stage = scratch.tile([P, T], f32, tag="scr")
                proj_kv(vT, wv_sb,
                        lambda tb, ps: nc.any.tensor_copy(
                            vt_stage[:, tb * 512:(tb + 1) * 512], ps[:]))
                for n in range(NT):
                    tp = psTP.tile([P, P], f32, tag="tp")
                    nc.tensor.transpose(
                        tp[:], vt_stage[:, n * P:(n + 1) * P], ident[:])
                    nc.vector.tensor_copy(vtm_sb[:, n * P:(n + 1) * P], tp[:])



                def l2normalize(xt, idx, gs_col):
                    """Scale columns of xt [128, T] by rsqrt(sum_d x^2)
                    (* per-head gain); partition sums via a ones-matmul,
                    small Newton chain in [16, 128] layout."""
                    sq = scratch.tile([P, T], MM_DT, tag="scr")
                    nc.vector.tensor_mul(sq[:], xt, xt)
                    for tb in range(TB):
                        ps = psTP.tile([1, 512], f32, tag="tp")
                        nc.tensor.matmul(
                            ps[:], ones[:],
                            sq[:, tb * 512:(tb + 1) * 512],
                            start=True, stop=True)
                        n2row = rows.tile([1, 512], f32, tag="n2row")
                        nc.vector.tensor_copy(n2row[:], ps[:])
                        nc.sync.dma_start(
                            n2_dram[idx:idx + 1, tb * 512:(tb + 1) * 512],
                            n2row[:])
                    n2c = smal.tile([NT, P], f32, tag="n2c")
                    nc.sync.dma_start(
                        n2c[:], n2_dram[idx, :].rearrange("(c p) -> c p", p=P))
                    # y = rsqrt(n2) with one Newton step:
                    # y0 = 1/sqrt(n2+eps); y1 = y0*(1.5 - 0.5*n2*y0^2)
                    sq_c = smal.tile([NT, P], f32, tag="sqc")
                    nc.scalar.activation(sq_c[:], n2c[:], AF.Sqrt,
                                         bias=eps16[:])
                    y0 = smal.tile([NT, P], f32, tag="y0")
                    nc.vector.reciprocal(y0[:], sq_c[:])
                    t1 = smal.tile([NT, P], f32, tag="t1")
                    nc.vector.tensor_mul(t1[:], y0[:], y0[:])
                    nc.vector.tensor_mul(t1[:], t1[:], n2c[:])
                    nc.vector.tensor_scalar(
                        out=t1[:], in0=t1[:], scalar1=-0.5, scalar2=1.5,
                        op0=mybir.AluOpType.mult, op1=mybir.AluOpType.add)
                    nc.vector.tensor_mul(y0[:], y0[:], t1[:])
                    if gs_col is not None:
                        nc.vector.tensor_mul(
                            y0[:], y0[:], gs_col.to_broadcast((NT, P)))
                    nc.sync.dma_start(
                        c_dram[idx, :].rearrange("(c p) -> c p", p=P), y0[:])
                    bc = bcast.tile([P, T], f32, tag="bc")
                    nc.sync.dma_start(
                        bc[:], c_dram[idx:idx + 1, :].to_broadcast((P, T)))
                    nc.vector.tensor_mul(xt, xt, bc[:])

                # Q projection: h-outer so each head's norm chain overlaps
                # the next head's matmuls; all 4 act blocks stay resident.
                qa = []
                for tb in range(TB):
                    a = actsq.tile([P, ND * 512], MM_DT, tag="acts",
                                   name=f"qa{tb}")
                    nc.sync.dma_start(a[:], _ld(qTt[tb]))
                    qa.append(a)
                for h in range(HPG):
                    for tb in range(TB):
                        ps = psA.tile([P, 512], f32, tag="proj")
                        for n in range(ND):
                            nc.tensor.matmul(
                                ps[:],
                                wq_sb[:, n * E + h * P:n * E + (h + 1) * P],
                                qa[tb][:, n * 512:(n + 1) * 512],
                                start=(n == 0), stop=(n == ND - 1))
                        nc.vector.tensor_copy(
                            qt_sb[:, h * T + tb * 512:
                                  h * T + (tb + 1) * 512], ps[:])
                    l2normalize(qt_sb[:, h * T:(h + 1) * T], h,
                                gs_sb[:, h:h + 1])

                wk_sb = wpool.tile([P, ND * DK], MM_DT, tag="wk")
                nc.sync.dma_start(wk_sb[:], _ld(wkt[:]))
                proj_kv(kT, wk_sb,
                        lambda tb, ps: nc.any.tensor_copy(
                            kt_sb[:, tb * 512:(tb + 1) * 512], ps[:]))
                l2normalize(kt_sb[:], HPG, None)

            # ------------- phase B+C: attention + out projection ----------
            atp = outer.enter_context(tc.tile_pool(name="atp", bufs=3))
            bcy = outer.enter_context(tc.tile_pool(name="bcy", bufs=2))
            invp = outer.enter_context(tc.tile_pool(name="invp", bufs=2))
            wo_pool = outer.enter_context(tc.tile_pool(name="wo", bufs=1))
            ostage = outer.enter_context(tc.tile_pool(name="ostage", bufs=3))
            ps_st = outer.enter_context(
                tc.tile_pool(name="ps_st", bufs=2, space="PSUM"))
            ps_y = outer.enter_context(
                tc.tile_pool(name="ps_y", bufs=1, space="PSUM"))
            ps_sums = outer.enter_context(
                tc.tile_pool(name="ps_sums", bufs=1, space="PSUM"))
            ps_o = outer.enter_context(
                tc.tile_pool(name="ps_o", bufs=2, space="PSUM"))

            wo_sb = wo_pool.tile([P, HPG * D], MM_DT, tag="wo")
            nc.sync.dma_start(wo_sb[:], _ld(wot[:]))

            for qb in range(TB):
                n_k = 4 * (qb + 1)
                for h in range(HPG):
                    qh = qt_sb[:, h * T + qb * 512:h * T + (qb + 1) * 512]
                    # stage 1: S^T -> exp -> SBUF strip (ACT-bound)
                    strip = atp.tile([P, NT * 512], MM_DT, tag="strip")
                    for kp in range(n_k // 2):
                        st = ps_st.tile([P, 1024], f32, tag="st")
                        for j2 in range(2):
                            kt = 2 * kp + j2
                            nc.tensor.matmul(
                                st[:, j2 * 512:(j2 + 1) * 512],
                                kt_sb[:, kt * P:(kt + 1) * P],
                                qh, start=True, stop=True)
                        ssl = strip[:, kp * 1024:(kp + 1) * 1024]
                        nc.scalar.activation(ssl, st[:], AF.Exp)
                        for j2 in range(2):
                            kt = 2 * kp + j2
                            j = kt - 4 * qb
                            if j >= 0:  # diagonal tile: causal mask
                                nc.vector.tensor_mul(
                                    strip[:, kt * 512:(kt + 1) * 512],
                                    strip[:, kt * 512:(kt + 1) * 512],
                                    maskM[:, 384 - j * P:896 - j * P])
                    # stage 2: dense rowsum + Y matmuls from the strip
                    ps_sm = ps_sums.tile([1, 512], f32, tag="sums")
                    for kt in range(n_k):
                        nc.tensor.matmul(
                            ps_sm[:], ones[:],
                            strip[:, kt * 512:(kt + 1) * 512],
                            start=(kt == 0), stop=(kt == n_k - 1))
                    inv_row = invp.tile([1, 512], f32, tag="inv")
                    nc.vector.reciprocal(inv_row[:], ps_sm[:])
                    nc.sync.dma_start(
                        inv_dram[h:h + 1, qb * 512:(qb + 1) * 512],
                        inv_row[:])
                    ps_yt = ps_y.tile([P, 512], f32, tag="y")
                    for kt in range(n_k):
                        nc.tensor.matmul(
                            ps_yt[:], vtm_sb[:, kt * P:(kt + 1) * P],
                            strip[:, kt * 512:(kt + 1) * 512],
                            start=(kt == 0), stop=(kt == n_k - 1))
                    yslice = yt_sb[:, h * T + qb * 512:h * T + (qb + 1) * 512]
                    nc.vector.tensor_copy(yslice, ps_yt[:])
                    bc = bcy.tile([P, 512], f32, tag="bcy")
                    nc.sync.dma_start(
                        bc[:], inv_dram[h:h + 1, qb * 512:(qb + 1) * 512]
                        .to_broadcast((P, 512)))
                    nc.vector.tensor_mul(yslice, yslice, bc[:])

                # out projection for this t-block (overlaps next q-block)
                tb = qb
                for ot in range(NT):
                    ps = ps_o.tile([P, 512], f32, tag="o")
                    for h in range(HPG):
                        nc.tensor.matmul(
                            ps[:],
                            wo_sb[:, h * D + ot * P:h * D + (ot + 1) * P],
                            yt_sb[:, h * T + tb * 512:h * T + (tb + 1) * 512],
                            start=(h == 0), stop=(h == HPG - 1))
                    o_sb = ostage.tile([P, 512], f32, tag="osb")
                    nc.any.tensor_copy(o_sb[:], ps[:])
                    nc.sync.dma_start(
                        outT[ot * P:(ot + 1) * P, tb * 512:(tb + 1) * 512],
                        o_sb[:])

    nc.compile()
    return nc


def make_in_maps(q, k, v, Wq, Wk, Wv, Wo, g):
    import ml_dtypes
    st = ml_dtypes.bfloat16 if ST_DT == bf16 else np.float32
    in_maps = []
    act_t = {}
    for b in range(B):
        qTb = np.ascontiguousarray(q[b].T).astype(st)
        # [TB, P, ND*512]: row p of block tb = concat_n qT[n*128+p, tb*512:]
        qTt = np.ascontiguousarray(
            qTb.reshape(ND, P, TB, 512).transpose(2, 1, 0, 3)
            .reshape(TB, P, ND * 512))
        act_t[b] = (
            qTt,
            np.ascontiguousarray(k[b].T).astype(st),
            np.ascontiguousarray(v[b].T).astype(st),
        )

    def wtile(wT, cols):  # wT: (D, cols) -> [P, ND*cols] row-tiled
        return np.ascontiguousarray(
            np.ascontiguousarray(wT).reshape(-1, P, cols)
            .transpose(1, 0, 2).reshape(P, -1)).astype(st)

    g_flat = np.asarray(g, dtype=np.float32).reshape(H)
    for c in range(8):
        b, gi = divmod(c, KVH)
        qTt, kTb, vTb = act_t[b]
        e0 = gi * E
        gvals = g_flat[gi * HPG:(gi + 1) * HPG] / math.sqrt(DK)
        in_maps.append({
            "qTt": qTt, "kT": kTb, "vT": vTb,
            "wqt": wtile(Wq[e0:e0 + E, :].T, E),
            "wkt": wtile(Wk[gi * DK:(gi + 1) * DK, :].T, DK),
            "wvt": wtile(Wv[gi * DK:(gi + 1) * DK, :].T, DK),
            "wot": wtile(Wo[:, e0:e0 + E].T, D),
            "gs16": np.broadcast_to(gvals[None, :], (NT, HPG)).copy(),
        })
    return in_maps


_cached = {}


def kernel(q, k, v, Wq, Wk, Wv, Wo, g, _trace=False, _tmpdir=None):
    if "nc" not in _cached:
        _cached["nc"] = build_kernel()
    nc = _cached["nc"]
    in_maps = make_in_maps(
        np.asarray(q, np.float32), np.asarray(k, np.float32),
        np.asarray(v, np.float32), np.asarray(Wq, np.float32),
        np.asarray(Wk, np.float32), np.asarray(Wv, np.float32),
        np.asarray(Wo, np.float32), g)
    res = run_bass_kernel_spmd(
        nc, in_maps, list(range(8)), trace=_trace, tmpdir=_tmpdir)
    out = np.empty((B, T, D), dtype=np.float32)
    for b in range(B):
        acc = res.results[4 * b]["outT"].copy()
        for gi in range(1, KVH):
            acc += res.results[4 * b + gi]["outT"]
        out[b] = acc.T
    kernel.last_results = res
    return out

